# revision 9
# baseline (speedup 1.0000x reference)
"""GNN message-passing kernel for 8 Trainium2 NeuronCores (Bass/Tile).

Takes FULL inputs, shards nodes across 8 cores internally, runs the
4-layer GNN (dense -> spmm -> spmm -> dense).  The hidden node tables
are AllGathered in three chunks (fired as soon as the producing blocks
finish) so the consumer spmm can start before the full table arrives.
Weighted segment sums run on the PE via one-hot selector matrices that
are built on-chip by the vector engine (batched stride-0 broadcast
compare+mult).  Row gathers go through dma_gather round-robined over 4
SWDGE queues so descriptor generation pipelines across the GpSimd Q7
core pairs.
"""

import math
from contextlib import ExitStack
from dataclasses import dataclass

import ml_dtypes
import numpy as np

import concourse.bass as bass
import concourse.mybir as mybir
import concourse.tile as tile
from concourse import bacc
from concourse.bass_utils import run_bass_kernel_spmd
from concourse.masks import make_identity

BF16 = ml_dtypes.bfloat16
AF = mybir.ActivationFunctionType
ALU = mybir.AluOpType

# table thirds, in 128-row blocks per core (must align to slab_blocks)
THIRDS = ((0, 16), (16, 32), (32, 49))


@dataclass(frozen=True)
class Cfg:
    n_nodes: int = 50000
    n_edges: int = 800000
    in_dim: int = 512
    h1: int = 512
    h2: int = 256
    out_dim: int = 128
    n_cores: int = 8
    slab_blocks: int = 4
    group_blocks: int = 2  # row-blocks per gather/P/psum group
    max_call_chunks: int = 15

    @property
    def nodes_per_core(self):
        return self.n_nodes // self.n_cores  # 6250

    @property
    def npad(self):
        return math.ceil(self.nodes_per_core / 128) * 128  # 6272

    @property
    def nblocks(self):
        return self.npad // 128  # 49

    def rows_t(self, t):
        lo, hi = THIRDS[t]
        return (hi - lo) * 128

    def tab_t(self, t):
        return self.rows_t(t) * self.n_cores


FULL = Cfg()


# ---------------------------------------------------------------- host prep


def edge_structure(cfg: Cfg, edge_row, edge_col, edge_weight):
    """Bucket edges per (core, row-block, table-third); uniform chunk counts.

    meta:
      nch[b][t]   chunks for block b, third t (same on all cores)
      off16[b][t] idx-tile int16-column offset of the bucket
      offch[b][t] global chunk index of the bucket
      groups      [(b0, b1)] row-block groups
      totch, idxcols
    per_core[c] = dict(idx, lrt, wt)
    """
    nc_, nb = cfg.n_cores, cfg.nblocks
    npc = cfg.nodes_per_core
    for t in range(3):
        assert cfg.tab_t(t) <= 32767

    core_of = edge_row // npc
    lr_all = edge_row - core_of * npc
    cc_of = edge_col // npc
    cl_all = edge_col - cc_of * npc
    blk_c = cl_all // 128
    third_all = np.searchsorted(
        np.array([THIRDS[0][1], THIRDS[1][1]]), blk_c, side="right"
    )
    t_lo = np.array([THIRDS[t][0] * 128 for t in range(3)])
    t_rows = np.array([cfg.rows_t(t) for t in range(3)])
    tidx_all = cc_of * t_rows[third_all] + (cl_all - t_lo[third_all])

    counts = np.zeros((nc_, nb, 3), np.int64)
    per = {}
    for c in range(nc_):
        m = core_of == c
        lr, ti, tt, w = lr_all[m], tidx_all[m], third_all[m], edge_weight[m]
        blk = lr // 128
        order = np.lexsort((ti, tt, blk))
        per[c] = (lr[order], ti[order], tt[order], w[order], blk[order])
        np.add.at(counts[c], (blk, tt), 1)

    chunks_bt = np.ceil(counts / 128.0).astype(np.int64).max(axis=0)
    chunks_bt = np.maximum(chunks_bt, 1)

    nch = [[int(chunks_bt[b, t]) for t in range(3)] for b in range(nb)]
    off16 = [[0, 0, 0] for _ in range(nb)]
    offch = [[0, 0, 0] for _ in range(nb)]
    tot16 = 0
    totch = 0
    for b in range(nb):
        for t in range(3):
            off16[b][t] = tot16
            offch[b][t] = totch
            tot16 += nch[b][t] * 8
            totch += nch[b][t]

    groups = []
    b0 = 0
    while b0 < nb:
        groups.append((b0, min(b0 + cfg.group_blocks, nb)))
        b0 = groups[-1][1]

    meta = dict(
        nch=nch, off16=off16, offch=offch, totch=totch, idxcols=tot16,
        groups=groups,
    )

    per_core = []
    for c in range(nc_):
        lr, ti, tt, w, blk = per[c]
        idx_flat = np.zeros(tot16 * 16, np.int16)
        lr_tab = np.zeros((128, totch), BF16)
        w_tab = np.zeros((128, totch), BF16)
        p = 0
        for b in range(nb):
            for t in range(3):
                q = p
                while q < len(blk) and blk[q] == b and tt[q] == t:
                    q += 1
                e_ti, e_lr, e_w = ti[p:q], lr[p:q], w[p:q]
                p = q
                n = len(e_ti)
                nslots = nch[b][t] * 128
                ti_pad = np.concatenate([e_ti, np.zeros(nslots - n, np.int64)])
                i_in = np.arange(nslots)
                base16 = off16[b][t]
                idx_flat[(base16 + i_in // 16) * 16 + (i_in % 16)] = (
                    ti_pad.astype(np.int16)
                )
                if n > 0:
                    j0 = offch[b][t]
                    i_e = np.arange(n)
                    jj = j0 + i_e // 128
                    ss = i_e % 128
                    lr_tab[ss, jj] = (e_lr - b * 128).astype(BF16)
                    w_tab[ss, jj] = e_w.astype(BF16)
        idx_mat = idx_flat.reshape(tot16, 16).T
        idx_mat = np.tile(idx_mat, (8, 1))
        per_core.append(
            dict(
                idx=np.ascontiguousarray(idx_mat),
                lrt=np.ascontiguousarray(lr_tab),
                wt=np.ascontiguousarray(w_tab),
            )
        )
    return meta, per_core


def prep_inputs(cfg: Cfg, inputs):
    f = inputs["features"].astype(np.float32)
    meta, per_edge = edge_structure(
        cfg,
        inputs["edge_row"].astype(np.int64),
        inputs["edge_col"].astype(np.int64),
        inputs["edge_weight"].astype(np.float32),
    )
    kin = cfg.in_dim // 128
    k1 = cfg.h1 // 128
    k2 = cfg.h2 // 128

    def wlayout(w, kt):
        K, M = w.shape
        return (
            w.reshape(kt, 128, M).transpose(1, 0, 2).reshape(128, kt * M)
        ).astype(BF16)

    w1 = wlayout(inputs["W_lin1"].astype(np.float32), kin)
    wg1 = wlayout(inputs["W_g1"].astype(np.float32), k1)
    wg2 = wlayout(inputs["W_g2"].astype(np.float32), k2)
    wl2 = wlayout(inputs["W_lin2"].astype(np.float32), k2)
    b1 = inputs["b_lin1"].astype(np.float32).reshape(kin, 128).T.copy()
    bg1 = inputs["b_g1"].astype(BF16).reshape(1, cfg.h2)
    bg2 = inputs["b_g2"].astype(BF16).reshape(1, cfg.h2)
    bl2 = inputs["b_lin2"].astype(BF16).reshape(1, cfg.out_dim)

    npc, npad = cfg.nodes_per_core, cfg.npad
    in_maps = []
    for c in range(cfg.n_cores):
        lo = c * npc
        hi = min((c + 1) * npc, cfg.n_nodes)
        xc = np.zeros((npad, cfg.in_dim), np.float32)
        xc[: hi - lo] = f[lo:hi]
        xt = (
            xc.T.reshape(kin, 128, npad)
            .transpose(1, 0, 2)
            .reshape(128, kin * npad)
        ).astype(BF16)
        in_maps.append(
            {
                "xt": np.ascontiguousarray(xt),
                "w1": w1,
                "wg1": wg1,
                "wg2": wg2,
                "wl2": wl2,
                "b1": b1,
                "bg1": bg1,
                "bg2": bg2,
                "bl2": bl2,
                "idx": per_edge[c]["idx"],
                "lrt": per_edge[c]["lrt"],
                "wt": per_edge[c]["wt"],
            }
        )
    return meta, in_maps


# ---------------------------------------------------------------- kernel IR


def build(cfg: Cfg, meta):
    nc = bacc.Bacc(
        "TRN2",
        target_bir_lowering=False,
        debug=False,
        num_devices=cfg.n_cores,
        num_swdge_queues=4,
    )
    bf = mybir.dt.bfloat16
    f32 = mybir.dt.float32
    i16 = mybir.dt.int16
    kin = cfg.in_dim // 128
    k1 = cfg.h1 // 128
    k2 = cfg.h2 // 128
    npad, nb, H2, OUT = cfg.npad, cfg.nblocks, cfg.h2, cfg.out_dim
    totch = meta["totch"]
    nch = meta["nch"]
    off16 = meta["off16"]
    offch = meta["offch"]
    groups = meta["groups"]

    xt_d = nc.dram_tensor("xt", [128, kin * npad], bf, kind="ExternalInput").ap()
    w1_d = nc.dram_tensor("w1", [128, kin * cfg.h1], bf, kind="ExternalInput").ap()
    wg1_d = nc.dram_tensor("wg1", [128, k1 * H2], bf, kind="ExternalInput").ap()
    wg2_d = nc.dram_tensor("wg2", [128, k2 * H2], bf, kind="ExternalInput").ap()
    wl2_d = nc.dram_tensor("wl2", [128, k2 * OUT], bf, kind="ExternalInput").ap()
    b1_d = nc.dram_tensor("b1", [128, kin], f32, kind="ExternalInput").ap()
    bg1_d = nc.dram_tensor("bg1", [1, H2], bf, kind="ExternalInput").ap()
    bg2_d = nc.dram_tensor("bg2", [1, H2], bf, kind="ExternalInput").ap()
    bl2_d = nc.dram_tensor("bl2", [1, OUT], bf, kind="ExternalInput").ap()
    idx_d = nc.dram_tensor(
        "idx", [128, meta["idxcols"]], i16, kind="ExternalInput"
    ).ap()
    lrt_d = nc.dram_tensor("lrt", [128, totch], bf, kind="ExternalInput").ap()
    wt_d = nc.dram_tensor("wt", [128, totch], bf, kind="ExternalInput").ap()
    y_d = nc.dram_tensor("y", [npad, OUT], f32, kind="ExternalOutput").ap()

    g1loc = [
        nc.dram_tensor(f"g1loc{t}", [cfg.rows_t(t), H2], bf).ap()
        for t in range(3)
    ]
    g2loc = [
        nc.dram_tensor(f"g2loc{t}", [cfg.rows_t(t), H2], bf).ap()
        for t in range(3)
    ]
    g1tab = [
        nc.dram_tensor(
            f"g1tab{t}", [cfg.tab_t(t), H2], bf, addr_space="Shared"
        ).ap()
        for t in range(3)
    ]
    g2tab = [
        nc.dram_tensor(
            f"g2tab{t}", [cfg.tab_t(t), H2], bf, addr_space="Shared"
        ).ap()
        for t in range(3)
    ]

    rg = [list(range(cfg.n_cores))]
    qctr = [0]

    def fire_ag(src, dst):
        nc.gpsimd.collective_compute(
            "AllGather",
            mybir.AluOpType.bypass,
            replica_groups=rg,
            ins=[src[:, :]],
            outs=[dst[:, :]],
        )

    t_of_block = {}
    for t, (lo, hi) in enumerate(THIRDS):
        for b in range(lo, hi):
            t_of_block[b] = (t, b - lo)

    with tile.TileContext(nc) as tc:
        with ExitStack() as top:
            const = top.enter_context(tc.tile_pool(name="const", bufs=1))
            w1_s = const.tile([128, kin * cfg.h1], bf)
            nc.sync.dma_start(w1_s[:], w1_d[:, :])
            wg1_s = const.tile([128, k1 * H2], bf)
            nc.sync.dma_start(wg1_s[:], wg1_d[:, :])
            wg2_s = const.tile([128, k2 * H2], bf)
            nc.sync.dma_start(wg2_s[:], wg2_d[:, :])
            wl2_s = const.tile([128, k2 * OUT], bf)
            nc.sync.dma_start(wl2_s[:], wl2_d[:, :])
            b1_s = const.tile([128, kin], f32)
            nc.sync.dma_start(b1_s[:], b1_d[:, :])
            bg1_s = const.tile([1, H2], bf)
            nc.sync.dma_start(bg1_s[:], bg1_d[:, :])
            bg2_s = const.tile([1, H2], bf)
            nc.sync.dma_start(bg2_s[:], bg2_d[:, :])
            bl2_s = const.tile([1, OUT], bf)
            nc.sync.dma_start(bl2_s[:], bl2_d[:, :])
            idx_s = const.tile([128, meta["idxcols"]], i16)
            nc.sync.dma_start(idx_s[:], idx_d[:, :])
            lrt_s = const.tile([128, totch], bf)
            nc.sync.dma_start(lrt_s[:], lrt_d[:, :])
            wt_s = const.tile([128, totch], bf)
            nc.sync.dma_start(wt_s[:], wt_d[:, :])
            iota_i = const.tile([128, 128], i16)
            nc.gpsimd.iota(
                iota_i[:], pattern=[[1, 128]], base=0, channel_multiplier=0
            )
            iota_b = const.tile([128, 128], bf)
            nc.vector.tensor_copy(iota_b[:], iota_i[:])
            ident = const.tile([128, 128], bf)
            make_identity(nc, ident[:])
            ones_t = const.tile([1, 128], bf)
            nc.gpsimd.memset(ones_t[:], 1.0)

            # ---------------- L1 + L2a, slab-streamed; fire AG1 thirds early
            slabs = []
            b0 = 0
            while b0 < nb:
                b1e = min(b0 + cfg.slab_blocks, nb)
                slabs.append((b0, b1e))
                b0 = b1e
            fire_after = {THIRDS[t][1]: t for t in range(3)}
            for t in range(3):
                assert THIRDS[t][1] in [be for (_, be) in slabs] or THIRDS[t][
                    1
                ] == nb

            with ExitStack() as pl1:
                xp = pl1.enter_context(tc.tile_pool(name="xt", bufs=2))
                hp = pl1.enter_context(tc.tile_pool(name="h1s", bufs=2))
                ps1 = pl1.enter_context(
                    tc.tile_pool(name="ps1", bufs=4, space="PSUM")
                )
                ps2 = pl1.enter_context(
                    tc.tile_pool(name="ps2", bufs=2, space="PSUM")
                )
                gp1 = pl1.enter_context(tc.tile_pool(name="g1t", bufs=3))
                for (bs, be) in slabs:
                    a = bs * 128
                    S = (be - bs) * 128
                    xs = xp.tile([128, kin, S], bf, tag="x")
                    for kt in range(kin):
                        nc.sync.dma_start(
                            xs[:, kt, :],
                            xt_d[:, kt * npad + a : kt * npad + a + S],
                        )
                    h1s = hp.tile([128, k1, S], bf, tag="h")
                    for f1t in range(k1):
                        ps = ps1.tile([128, S], f32, tag="ps")
                        for kt in range(kin):
                            nc.tensor.matmul(
                                ps[:],
                                lhsT=w1_s[
                                    :,
                                    kt * cfg.h1 + f1t * 128 : kt * cfg.h1
                                    + f1t * 128
                                    + 128,
                                ],
                                rhs=xs[:, kt, :],
                                start=(kt == 0),
                                stop=(kt == kin - 1),
                            )
                        nc.scalar.activation(
                            h1s[:, f1t, :],
                            ps[:],
                            AF.Sigmoid,
                            bias=b1_s[:, f1t : f1t + 1],
                        )
                    for b in range(bs, be):
                        o = (b - bs) * 128
                        ps = ps2.tile([128, H2], f32, tag="ps")
                        for kt in range(k1):
                            nc.tensor.matmul(
                                ps[:],
                                lhsT=h1s[:, kt, o : o + 128],
                                rhs=wg1_s[:, kt * H2 : (kt + 1) * H2],
                                start=(kt == 0),
                                stop=(kt == k1 - 1),
                            )
                        g1t = gp1.tile([128, H2], bf, tag="g1")
                        nc.vector.tensor_copy(g1t[:], ps[:])
                        t, bb = t_of_block[b]
                        nc.sync.dma_start(
                            g1loc[t][bb * 128 : (bb + 1) * 128, :], g1t[:]
                        )
                    if be in fire_after:
                        t = fire_after[be]
                        fire_ag(g1loc[t], g1tab[t])

            # ---------------- shared spmm machinery (pools shared across
            # both layers)
            MAXC = cfg.max_call_chunks
            grp_ch = [
                [sum(nch[b][t] for b in range(b0, b1)) for t in range(3)]
                for (b0, b1) in groups
            ]
            gmax = max(max(g) for g in grp_ch)
            ptmax = max(sum(g) for g in grp_ch)
            gp = top.enter_context(tc.tile_pool(name="gath", bufs=8))
            pp = top.enter_context(tc.tile_pool(name="pm", bufs=3))
            sp = top.enter_context(
                tc.tile_pool(name="psmm", bufs=4, space="PSUM")
            )

            def spmm_layer(ctx, tabs, brow, out_cb, tag):
                for gi, (b0, b1) in enumerate(groups):
                    tiles = {}
                    for t in range(3):
                        tile_g = gp.tile([128, gmax, H2], bf, tag="g")
                        pos = 0
                        for b in range(b0, b1):
                            n = nch[b][t]
                            for lo in range(0, n, MAXC):
                                ns = min(MAXC, n - lo)
                                o16 = off16[b][t] + lo * 8
                                nc.gpsimd.dma_gather(
                                    out_ap=tile_g[:, pos : pos + ns, :],
                                    in_ap=tabs[t][:, :],
                                    idxs_ap=idx_s[:, o16 : o16 + ns * 8],
                                    num_idxs=ns * 128,
                                    num_idxs_reg=ns * 128,
                                    elem_size=H2,
                                    single_packet=False,
                                    queue_num=qctr[0] % 4,
                                )
                                qctr[0] += 1
                                pos += ns
                        tiles[t] = tile_g
                    # batched P build for the whole group (chunk ids of the
                    # group's buckets are contiguous in block-major order)
                    j0 = offch[b0][0]
                    nct = sum(grp_ch[gi])
                    pt = pp.tile([128, ptmax, 128], bf, tag="p")
                    nc.vector.tensor_tensor(
                        pt[:, :nct, :],
                        iota_b[:].unsqueeze(1).broadcast_to((128, nct, 128)),
                        lrt_s[:, j0 : j0 + nct]
                        .unsqueeze(2)
                        .broadcast_to((128, nct, 128)),
                        ALU.is_equal,
                    )
                    nc.vector.tensor_tensor(
                        pt[:, :nct, :],
                        pt[:, :nct, :],
                        wt_s[:, j0 : j0 + nct]
                        .unsqueeze(2)
                        .broadcast_to((128, nct, 128)),
                        ALU.mult,
                    )
                    for b in range(b0, b1):
                        ps = sp.tile(
                            [128, H2], f32, tag="ps", name=f"psmm{tag}_{b}"
                        )
                        first = True
                        for t in range(3):
                            jl = offch[b][t] - j0
                            gpos = sum(nch[bb][t] for bb in range(b0, b))
                            for j in range(nch[b][t]):
                                nc.tensor.matmul(
                                    ps[:],
                                    lhsT=pt[:, jl + j, :],
                                    rhs=tiles[t][:, gpos + j, :],
                                    start=first,
                                    stop=False,
                                )
                                first = False
                        nc.tensor.matmul(
                            ps[:],
                            lhsT=ones_t[:1, :],
                            rhs=brow[:1, :],
                            start=first,
                            stop=True,
                        )
                        out_cb(b, ps)

            # ---------------- spmm1 + L3a fused per block; fire AG2 thirds
            with ExitStack() as s1:
                tps3 = s1.enter_context(
                    tc.tile_pool(name="tps3", bufs=2, space="PSUM")
                )
                psp3 = s1.enter_context(
                    tc.tile_pool(name="ps3", bufs=2, space="PSUM")
                )
                tp3 = s1.enter_context(tc.tile_pool(name="l3t", bufs=3))

                def cb1(b, psum):
                    h2t = tp3.tile([128, H2], bf, tag="h2")
                    nc.scalar.activation(h2t[:], psum[:], AF.Relu)
                    h2T = tp3.tile([128, k2, 128], bf, tag="h2T")
                    for kt in range(k2):
                        ptt = tps3.tile([128, 128], bf, tag="pt")
                        nc.tensor.transpose(
                            ptt[:], h2t[:, kt * 128 : (kt + 1) * 128], ident[:]
                        )
                        nc.vector.tensor_copy(h2T[:, kt, :], ptt[:])
                    ps3 = psp3.tile([128, H2], f32, tag="ps")
                    for kt in range(k2):
                        nc.tensor.matmul(
                            ps3[:],
                            lhsT=h2T[:, kt, :],
                            rhs=wg2_s[:, kt * H2 : (kt + 1) * H2],
                            start=(kt == 0),
                            stop=(kt == k2 - 1),
                        )
                    g2t = tp3.tile([128, H2], bf, tag="g2")
                    nc.vector.tensor_copy(g2t[:], ps3[:])
                    t, bb = t_of_block[b]
                    nc.sync.dma_start(
                        g2loc[t][bb * 128 : (bb + 1) * 128, :], g2t[:]
                    )
                    if b + 1 == THIRDS[t][1]:
                        fire_ag(g2loc[t], g2tab[t])

                spmm_layer(s1, g1tab, bg1_s, cb1, "a")

            # ---------------- spmm2 + L4 fused per block
            with ExitStack() as s2:
                tps4 = s2.enter_context(
                    tc.tile_pool(name="tps4", bufs=2, space="PSUM")
                )
                psp4 = s2.enter_context(
                    tc.tile_pool(name="ps4", bufs=2, space="PSUM")
                )
                tp4 = s2.enter_context(tc.tile_pool(name="l4t", bufs=3))

                def cb2(b, psum):
                    h3t = tp4.tile([128, H2], bf, tag="h3")
                    nc.scalar.activation(h3t[:], psum[:], AF.Relu)
                    h3T = tp4.tile([128, k2, 128], bf, tag="h3T")
                    for kt in range(k2):
                        ptt = tps4.tile([128, 128], bf, tag="pt")
                        nc.tensor.transpose(
                            ptt[:], h3t[:, kt * 128 : (kt + 1) * 128], ident[:]
                        )
                        nc.vector.tensor_copy(h3T[:, kt, :], ptt[:])
                    ps4 = psp4.tile([128, OUT], f32, tag="ps")
                    for kt in range(k2):
                        nc.tensor.matmul(
                            ps4[:],
                            lhsT=h3T[:, kt, :],
                            rhs=wl2_s[:, kt * OUT : (kt + 1) * OUT],
                            start=(kt == 0),
                            stop=False,
                        )
                    nc.tensor.matmul(
                        ps4[:],
                        lhsT=ones_t[:1, :],
                        rhs=bl2_s[:1, :],
                        start=False,
                        stop=True,
                    )
                    yt = tp4.tile([128, OUT], f32, tag="y")
                    nc.vector.tensor_copy(yt[:], ps4[:])
                    nc.sync.dma_start(y_d[b * 128 : (b + 1) * 128, :], yt[:])

                spmm_layer(s2, g2tab, bg2_s, cb2, "b")

    nc.compile()
    return nc


# ---------------------------------------------------------------- driver

_CACHE = {}


def run(inputs, cfg: Cfg = FULL, trace=False, tmpdir=None):
    meta, in_maps = prep_inputs(cfg, inputs)
    key = (cfg, meta["totch"], meta["idxcols"])
    if key not in _CACHE:
        _CACHE[key] = build(cfg, meta)
    nc = _CACHE[key]
    res = run_bass_kernel_spmd(
        nc,
        in_maps,
        core_ids=list(range(cfg.n_cores)),
        trace=trace,
        tmpdir=tmpdir,
    )
    npc = cfg.nodes_per_core
    out = np.empty((cfg.n_nodes, cfg.out_dim), np.float32)
    for c in range(cfg.n_cores):
        lo = c * npc
        hi = min((c + 1) * npc, cfg.n_nodes)
        out[lo:hi] = res.results[c]["y"][: hi - lo]
    return out, res


def kernel(**inputs) -> np.ndarray:
    out, _ = run(inputs, FULL, trace=False)
    return out
